# revision 8
# baseline (speedup 1.0000x reference)
"""Bass/Trainium2 kernel for nn_DiagonalTransfer.

Math: out[i, k] = logsumexp_j(D[i, j] + xx[j, k]) with D = diag(diag)
(zeros off-diagonal).  Since D is diagonal plus a zero background:

    out[i, k] = log( sum_j exp(xx[j, k]) + exp(xx[i, k]) * (exp(diag[i]) - 1) )
              = log( S[k] + E[i, k] * c[i] )

with S[k] = sum_j exp(xx[j, k]), E = exp(xx), c = expm1(diag).
All terms rewritten this way stay positive: S - E[i,k] >= sum_{j != i} E[j,k] > 0.

Device strategy (8 cores, data parallel over the K observation dim):
  - Host computes c = expm1(diag) and transposes xx -> xxT (K, N) so each
    core receives a contiguous (K/8, N) shard with k on partitions.
  - Per [128, N] tile: ScalarE Exp with accum_out yields E and the
    per-partition row sums S[k] in one pass; VectorE multiplies by the
    broadcast c row; ScalarE Ln with bias=S fuses the add and the log.
  - Output is the transposed shard; host re-transposes and concatenates.
"""

import numpy as np

import concourse.bass as bass
import concourse.bacc as bacc
import concourse.tile as tile
from concourse import mybir
from concourse.bass_utils import run_bass_kernel_spmd

N = 1024          # num_states (rows of xx, length of diag)
K = 8192          # observation columns of xx
NCORES = 8
KS = K // NCORES  # columns per core
P = 128           # SBUF partitions
NT = KS // P      # k-tiles per core

_cached_nc = None


def build_bass():
    """Per-core program: xxT shard (KS, N) + c (N,) -> outT shard (KS, N)."""
    nc = bacc.Bacc("TRN2", target_bir_lowering=False, debug=False)
    xxT = nc.declare_dram_parameter("xxT", [KS, N], mybir.dt.float32, isOutput=False)
    cvec = nc.declare_dram_parameter("c", [N], mybir.dt.float32, isOutput=False)
    outT = nc.declare_dram_parameter("outT", [KS, N], mybir.dt.float32, isOutput=True)

    # k-tiles are grouped into per-DMA batches.  Small batches at the start
    # ramp the pipeline quickly (the first EXP can begin as soon as the first
    # 512 KiB lands instead of waiting on a megabyte), and a small final
    # batch shortens the store tail.  SBUF batch tile is [128, B, N] where
    # chunk j of partition p holds DRAM row (base + j)*128 + p.
    BATCHES = [1, 1, 2, 2, 1, 1]
    assert sum(BATCHES) == NT
    BMAX = max(BATCHES)

    with tile.TileContext(nc) as tc:
        with (
            tc.tile_pool(name="const", bufs=1) as const_pool,
            tc.tile_pool(name="loads", bufs=6) as loads,
            tc.tile_pool(name="work", bufs=4) as work,
            tc.tile_pool(name="sums", bufs=8) as sums,
            tc.tile_pool(name="outs", bufs=4) as outs,
        ):
            # Preload the combined exp+ln activation table set so the
            # alternating Exp/Ln stream needs no per-tile table reloads.
            # act_func_set_id 6 == "natural_log_exp_and_others" for gen3.
            with tc.high_priority():
                nc.scalar.add_instruction(
                    mybir.InstLoadActFuncSet(
                        name=nc.get_next_instruction_name(),
                        ins=[],
                        outs=[],
                        act_func_set_id=6,
                    )
                )

            # c broadcast to all partitions once ([P, 1, N]; the multiply
            # reads it through a 0-step AP).  SWDGE keeps it off the SP ring
            # that streams the loads.
            c_b = const_pool.tile([P, 1, N], mybir.dt.float32)
            c_ap = cvec[:]
            c_src = bass.AP(
                tensor=c_ap.tensor, offset=c_ap.offset, ap=[[0, P], [0, 1], [1, N]]
            )
            nc.gpsimd.dma_start(out=c_b[:], in_=c_src)

            xxT_t = xxT.rearrange("(nt p) n -> nt p n", p=P)
            outT_t = outT.rearrange("(nt p) n -> nt p n", p=P)

            base = 0
            for bsz in BATCHES:
                x_t = loads.tile([P, bsz, N], mybir.dt.float32, tag="x")
                src = xxT_t[base : base + bsz].rearrange("b p n -> p b n")
                nc.sync.dma_start(out=x_t[:], in_=src)

                e_t = work.tile([P, bsz, N], mybir.dt.float32, tag="e")
                s_t = sums.tile([P, BMAX], mybir.dt.float32, tag="s")
                # E = exp(x); accum_out gives S[k] = sum_i E[k, i] per
                # partition. One activation per chunk: the accumulator must
                # not mix k-rows that share a partition.
                for j in range(bsz):
                    nc.scalar.activation(
                        out=e_t[:, j, :],
                        in_=x_t[:, j, :],
                        func=mybir.ActivationFunctionType.Exp,
                        accum_out=s_t[:, j : j + 1],
                    )
                # E *= c (broadcast along partitions and chunks)
                nc.vector.tensor_mul(
                    out=e_t[:], in0=e_t[:], in1=c_b[:].to_broadcast([P, bsz, N])
                )
                # out = ln(E*c + S)
                o_t = outs.tile([P, bsz, N], mybir.dt.float32, tag="o")
                for j in range(bsz):
                    nc.scalar.activation(
                        out=o_t[:, j, :],
                        in_=e_t[:, j, :],
                        func=mybir.ActivationFunctionType.Ln,
                        bias=s_t[:, j : j + 1],
                        scale=1.0,
                    )
                dst = outT_t[base : base + bsz].rearrange("b p n -> p b n")
                nc.gpsimd.dma_start(out=dst, in_=o_t[:])
                base += bsz
    nc.compile()
    return nc


def _get_nc():
    global _cached_nc
    if _cached_nc is None:
        _cached_nc = build_bass()
    return _cached_nc


def run(diag, xx, **spmd_kwargs):
    """Run on 8 cores; returns (out, BassKernelResults)."""
    diag = np.asarray(diag, dtype=np.float32)
    xx = np.asarray(xx, dtype=np.float32)
    c = np.expm1(diag.astype(np.float64)).astype(np.float32)
    xxT = np.ascontiguousarray(xx.T)  # (K, N)
    in_maps = [
        {"xxT": np.ascontiguousarray(xxT[i * KS : (i + 1) * KS]), "c": c}
        for i in range(NCORES)
    ]
    res = run_bass_kernel_spmd(_get_nc(), in_maps, list(range(NCORES)), **spmd_kwargs)
    outT = np.concatenate([res.results[i]["outT"] for i in range(NCORES)], axis=0)
    out = np.ascontiguousarray(outT.T).astype(np.float32)
    return out, res


def kernel(diag, xx):
    out, _ = run(diag, xx)
    return out


# revision 10
# speedup vs baseline: 1.0192x; 1.0192x over previous
"""Bass/Trainium2 kernel for nn_DiagonalTransfer.

Math: out[i, k] = logsumexp_j(D[i, j] + xx[j, k]) with D = diag(diag)
(zeros off-diagonal).  Since D is diagonal plus a zero background:

    out[i, k] = log( sum_j exp(xx[j, k]) + exp(xx[i, k]) * (exp(diag[i]) - 1) )
              = log( S[k] + E[i, k] * c[i] )

with S[k] = sum_j exp(xx[j, k]), E = exp(xx), c = expm1(diag).
All terms rewritten this way stay positive: S - E[i,k] >= sum_{j != i} E[j,k] > 0.

Device strategy (8 cores, data parallel over the K observation dim):
  - Host computes c = expm1(diag) and transposes xx -> xxT (K, N) so each
    core receives a contiguous (K/8, N) shard with k on partitions.
  - Per [128, N] tile: ScalarE Exp with accum_out yields E and the
    per-partition row sums S[k] in one pass; VectorE multiplies by the
    broadcast c row; ScalarE Ln with bias=S fuses the add and the log.
  - Output is the transposed shard; host re-transposes and concatenates.
"""

import numpy as np

import concourse.bass as bass
import concourse.bacc as bacc
import concourse.tile as tile
from concourse import mybir
from concourse.bass_utils import run_bass_kernel_spmd

N = 1024          # num_states (rows of xx, length of diag)
K = 8192          # observation columns of xx
NCORES = 8
KS = K // NCORES  # columns per core
P = 128           # SBUF partitions
NT = KS // P      # k-tiles per core

_cached_nc = None


def build_bass():
    """Per-core program: xxT shard (KS, N) + c (N,) -> outT shard (KS, N)."""
    nc = bacc.Bacc("TRN2", target_bir_lowering=False, debug=False)
    xxT = nc.declare_dram_parameter("xxT", [KS, N], mybir.dt.float32, isOutput=False)
    cvec = nc.declare_dram_parameter("c", [N], mybir.dt.float32, isOutput=False)
    outT = nc.declare_dram_parameter("outT", [KS, N], mybir.dt.float32, isOutput=True)

    # k-tiles are grouped into per-DMA batches.  Small batches at the start
    # ramp the pipeline quickly (the first EXP can begin as soon as the first
    # 512 KiB lands instead of waiting on a megabyte), and a small final
    # batch shortens the store tail.  SBUF batch tile is [128, B, N] where
    # chunk j of partition p holds DRAM row (base + j)*128 + p.
    BATCHES = [1, 1, 1, 1, 2, 2]
    assert sum(BATCHES) == NT
    BMAX = max(BATCHES)

    with tile.TileContext(nc) as tc:
        with (
            tc.tile_pool(name="const", bufs=1) as const_pool,
            tc.tile_pool(name="cpsum", bufs=1, space="PSUM") as cpsum,
            tc.tile_pool(name="loads", bufs=6) as loads,
            tc.tile_pool(name="work", bufs=4) as work,
            tc.tile_pool(name="sums", bufs=8) as sums,
            tc.tile_pool(name="outs", bufs=4) as outs,
        ):
            # Preload the combined exp+ln activation table set so the
            # alternating Exp/Ln stream needs no per-tile table reloads.
            # act_func_set_id 6 == "natural_log_exp_and_others" for gen3.
            with tc.high_priority():
                nc.scalar.add_instruction(
                    mybir.InstLoadActFuncSet(
                        name=nc.get_next_instruction_name(),
                        ins=[],
                        outs=[],
                        act_func_set_id=6,
                    )
                )

            xxT_t = xxT.rearrange("(nt p) n -> nt p n", p=P)
            outT_t = outT.rearrange("(nt p) n -> nt p n", p=P)

            # First input batch gets the SP ring to itself before anything
            # else touches the DMA engines.
            x_tiles = []
            bases = []
            base = 0
            for bi, bsz in enumerate(BATCHES):
                x_t = loads.tile([P, bsz, N], mybir.dt.float32, tag="x")
                src = xxT_t[base : base + bsz].rearrange("b p n -> p b n")
                nc.sync.dma_start(out=x_t[:], in_=src)
                x_tiles.append(x_t)
                bases.append(base)
                base += bsz
                if bi == 0:
                    # c rides in as a single 4 KiB row, then the (otherwise
                    # idle) TensorE replicates it to all 128 partitions in
                    # PSUM, where the multiply reads it directly.
                    c_row = const_pool.tile([1, N], mybir.dt.float32)
                    nc.sync.dma_start(out=c_row[:], in_=cvec[:][None, :])
                    ones = const_pool.tile([1, P], mybir.dt.float32)
                    nc.vector.memset(ones[:], 1.0)
                    c_b = cpsum.tile([P, 1, N], mybir.dt.float32)
                    # one matmul per PSUM bank (N<=512 fp32 limit)
                    for h in range(0, N, 512):
                        nc.tensor.matmul(
                            c_b[:, 0, h : h + 512],
                            ones[:],
                            c_row[:, h : h + 512],
                            start=True,
                            stop=True,
                        )

            for bi, bsz in enumerate(BATCHES):
                x_t = x_tiles[bi]
                e_t = work.tile([P, bsz, N], mybir.dt.float32, tag="e")
                s_t = sums.tile([P, BMAX], mybir.dt.float32, tag="s")
                # E = exp(x); accum_out gives S[k] = sum_i E[k, i] per
                # partition. One activation per chunk: the accumulator must
                # not mix k-rows that share a partition.
                for j in range(bsz):
                    nc.scalar.activation(
                        out=e_t[:, j, :],
                        in_=x_t[:, j, :],
                        func=mybir.ActivationFunctionType.Exp,
                        accum_out=s_t[:, j : j + 1],
                    )
                # E *= c (broadcast along partitions and chunks)
                nc.vector.tensor_mul(
                    out=e_t[:], in0=e_t[:], in1=c_b[:].to_broadcast([P, bsz, N])
                )
                # out = ln(E*c + S)
                o_t = outs.tile([P, bsz, N], mybir.dt.float32, tag="o")
                for j in range(bsz):
                    nc.scalar.activation(
                        out=o_t[:, j, :],
                        in_=e_t[:, j, :],
                        func=mybir.ActivationFunctionType.Ln,
                        bias=s_t[:, j : j + 1],
                        scale=1.0,
                    )
                dst = outT_t[bases[bi] : bases[bi] + bsz].rearrange("b p n -> p b n")
                nc.gpsimd.dma_start(out=dst, in_=o_t[:])
    nc.compile()
    return nc


def _get_nc():
    global _cached_nc
    if _cached_nc is None:
        _cached_nc = build_bass()
    return _cached_nc


def run(diag, xx, **spmd_kwargs):
    """Run on 8 cores; returns (out, BassKernelResults)."""
    diag = np.asarray(diag, dtype=np.float32)
    xx = np.asarray(xx, dtype=np.float32)
    c = np.expm1(diag.astype(np.float64)).astype(np.float32)
    xxT = np.ascontiguousarray(xx.T)  # (K, N)
    in_maps = [
        {"xxT": np.ascontiguousarray(xxT[i * KS : (i + 1) * KS]), "c": c}
        for i in range(NCORES)
    ]
    res = run_bass_kernel_spmd(_get_nc(), in_maps, list(range(NCORES)), **spmd_kwargs)
    outT = np.concatenate([res.results[i]["outT"] for i in range(NCORES)], axis=0)
    out = np.ascontiguousarray(outT.T).astype(np.float32)
    return out, res


def kernel(diag, xx):
    out, _ = run(diag, xx)
    return out
